# revision 27
# baseline (speedup 1.0000x reference)
"""Causal self-attention (B=4, T=2048, C=1024, H=16, Dh=64) on 8 trn2 NeuronCores.

Sharding: core = 2*b + g  (b = batch 0..3, g = head-group 0..1, 8 heads each).
Each core computes its batch's QKV projection for its 8 heads, causal
attention, and a partial out-projection; host sums the two head-group
partials per batch (the "all-reduce" of the tensor-parallel split).

v2 scheduling (vs v1): the whole kernel is organized so the PE never
idles and the tail never crams:
  - inputs split into per-ct (128-row) DMAs issued in need order across
    all five engine queues, so the first chase matmuls start ~10us in
    instead of ~24us (whole-tensor DMAs made the chase wait on full wv).
  - startup "chase": two groups of 8 parallel PSUM accumulation chains
    (V-proj + q/k tt0/tt1 chunks) consume each xt ct-tile as it lands.
  - attention runs J-outer / m-inner (v1 was m-outer), so the out
    projection for tq block J unlocks after J's last head pair instead
    of after 75% of the kernel; out-proj chunks become PE filler spread
    through the attention phase, shrinking the drain tail.
  - one K=2 selector matmul broadcasts both head rowsums per block
    (v1: two K=1 matmuls), halving norm PE rows.
Everything else (bf16 matmuls, ones-column rowsum in PV, trimmed exp
slabs, GpSimd triangular masking, PV lagged 3 slabs, deferred norm) is
as in v1.
"""

import sys

for _p in ("/opt/trn_rl_repo", "/opt/pypackages"):
    if _p not in sys.path:
        sys.path.append(_p)

import numpy as np
from collections import deque
from contextlib import ExitStack

import concourse.bass as bass
import concourse.tile as tile
from concourse import bacc, mybir
from concourse.bass_utils import run_bass_kernel_spmd

import ml_dtypes

BF16NP = np.dtype(ml_dtypes.bfloat16)

B, T, C = 4, 2048, 1024
H, DH = 16, 64
HG = 8          # heads per core
JW = 512        # tq tile width
NT = T // JW    # 4 tq tiles
NK = T // 128   # 16 tk tiles
F32 = mybir.dt.float32
BF = mybir.dt.bfloat16
EXP = mybir.ActivationFunctionType.Exp

_cache = {}


def _build():
    nc = bacc.Bacc("TRN2", target_bir_lowering=False, debug=False, num_devices=8)
    xT = nc.dram_tensor("xT", [C, T], BF, kind="ExternalInput").ap()
    wqk = nc.dram_tensor("wqk", [C, 1024], BF, kind="ExternalInput").ap()
    wv = nc.dram_tensor("wv", [C, 512], BF, kind="ExternalInput").ap()
    wout = nc.dram_tensor("wout", [512, C], BF, kind="ExternalInput").ap()
    trimask = nc.dram_tensor("trimask", [128, 128], BF, kind="ExternalInput").ap()
    sel = nc.dram_tensor("sel", [2, 128], BF, kind="ExternalInput").ap()
    out = nc.dram_tensor("out", [T, C], BF, kind="ExternalOutput").ap()

    with tile.TileContext(nc) as tc:
        with ExitStack() as ctx:
            ctx.enter_context(nc.allow_low_precision(reason="bf16 matmuls intended"))
            sb = ctx.enter_context(tc.tile_pool(name="sb", bufs=1))
            ppool = ctx.enter_context(tc.tile_pool(name="ppool", bufs=6))
            small = ctx.enter_context(tc.tile_pool(name="small", bufs=2))
            otp = ctx.enter_context(tc.tile_pool(name="otp", bufs=2))
            # PSUM: slab 2x2 banks + psy 2 + bc 1 + pj 1 = 8 banks exactly
            slab_p = ctx.enter_context(tc.tile_pool(name="slab_p", bufs=2, space="PSUM"))
            psy_p = ctx.enter_context(tc.tile_pool(name="psy_p", bufs=2, space="PSUM"))
            bc_p = ctx.enter_context(tc.tile_pool(name="bc_p", bufs=1, space="PSUM"))
            pj_p = ctx.enter_context(tc.tile_pool(name="pj_p", bufs=1, space="PSUM"))

            # ---- persistent SBUF (per-ct tiles: DMA dependency is per tile,
            # so consumers start as soon as their 128-row slice lands) ----
            # xt per ct, split into t-halves: the chase (V it0-7, q/k
            # tt0/tt1) reads only half A, so it lands ~7us sooner than a
            # whole [128, 2048] tile would
            xtA = [sb.tile([128, T // 2], BF, tag=f"xtA{ct}", name=f"xtA{ct}")
                   for ct in range(8)]
            xtB = [sb.tile([128, T // 2], BF, tag=f"xtB{ct}", name=f"xtB{ct}")
                   for ct in range(8)]
            wv_t = [sb.tile([128, 512], BF, tag=f"wv{ct}", name=f"wv{ct}")
                    for ct in range(8)]
            wqk_t = [sb.tile([128, 1024], BF, tag=f"wqk{ct}", name=f"wqk{ct}")
                     for ct in range(8)]
            wout_t = [sb.tile([128, 1024], BF, tag=f"wout{jt}", name=f"wout{jt}")
                      for jt in range(4)]
            qk_sb = [sb.tile([128, T], BF, tag=f"qk{j}", name=f"qk{j}") for j in range(8)]
            v_all = sb.tile([128, NK * HG * 65], BF, tag="v")
            y_sb = [sb.tile([128, T], BF, tag=f"y{m}", name=f"y{m}") for m in range(4)]
            tri_sb = sb.tile([128, 128], BF, tag="tri")
            sel_sb = sb.tile([2, 128], BF, tag="sel")
            scratch = sb.tile([1, 64], BF, tag="scratch")

            def xv(ct, it):  # V-proj stationary slice (128 t-cols)
                t = xtA[ct] if it < 8 else xtB[ct]
                return t[:, 128 * (it % 8):128 * (it % 8) + 128]

            def xq(ct, tt):  # q/k-proj moving slice (JW t-cols)
                t = xtA[ct] if tt < 2 else xtB[ct]
                return t[:, JW * (tt % 2):JW * (tt % 2) + JW]

            wvt = lambda ct: wv_t[ct][:]
            wqkt = lambda ct, jt: wqk_t[ct][:, 128 * jt:128 * jt + 128]
            woutt = lambda jt, et: wout_t[jt][:, 512 * et:512 * et + 512]
            vt = lambda i: v_all[:, 520 * i:520 * (i + 1)]

            # ---- input DMAs: need-ordered across the 3 DMA-capable queues
            # (sync/SP, scalar/ACT, gpsimd). Service is roughly FIFO by
            # issue time, so interleave wv (chase V chains) with xt pairs
            # on the two fast-starting queues; wqk (first needed ~chase
            # group G2) queues behind xt on scalar; wout + masks (needed
            # last) on gpsimd ----
            TH = T // 2
            for k in range(4):
                ct = 2 * k
                nc.sync.dma_start(xtA[ct][:], xT[128 * ct:128 * ct + 128, 0:TH])
                nc.sync.dma_start(wv_t[ct][:], wv[128 * ct:128 * ct + 128, :])
            for k in range(4):
                ct = 2 * k + 1
                nc.scalar.dma_start(xtA[ct][:], xT[128 * ct:128 * ct + 128, 0:TH])
                nc.scalar.dma_start(wv_t[ct][:], wv[128 * ct:128 * ct + 128, :])
            for k in range(4):
                nc.sync.dma_start(xtB[2 * k][:], xT[256 * k:256 * k + 128, TH:T])
                nc.scalar.dma_start(xtB[2 * k + 1][:],
                                    xT[256 * k + 128:256 * k + 256, TH:T])
            for ct in range(8):
                nc.gpsimd.dma_start(wqk_t[ct][:], wqk[128 * ct:128 * ct + 128, :])
            nc.gpsimd.dma_start(tri_sb[:], trimask[:])
            nc.gpsimd.dma_start(sel_sb[:], sel[:])
            for jt in range(4):
                nc.gpsimd.dma_start(wout_t[jt][:], wout[128 * jt:128 * jt + 128, :])
            # preload the exp table set while DMAs run
            nc.scalar.activation(scratch[:], sel_sb[0:1, 0:64], EXP, scale=0.125)
            # only the ones-columns (65th of every 65-wide head slice) need
            # setting; strided memset is ~50x cheaper than filling all of v
            nc.vector.memset(
                v_all[:].rearrange("p (x d) -> p x d", d=65)[:, :, 64:65], 1.0)

            # ---- startup chase: two groups of 8 parallel accumulation
            # chains consume each xt ct-tile as its DMA lands (8 matmuls per
            # arriving tile), instead of the PE idling through the ~25us
            # input transfer window. V group first (needs only wv+xt, the
            # earliest arrivals); the q/k tt0 group after (needs wqk,
            # arriving behind xt). The chase covers ONLY the DMA window:
            # a longer stretch of dense full-array projection matmuls
            # trips the HAM 50%-utilization power throttle (ham type-1),
            # so the rest of the projections stay interleaved with the
            # lower-power K=64 attention slabs as filler.
            CHASE = [
                [("v", it) for it in range(0, 8)],
                [("p", jt, 0) for jt in (0, 4, 1, 5, 2, 6, 3, 7)],
            ]
            for group in CHASE:
                slabA = slab_p.tile([128, 1024], F32, tag="slab", name="chA")
                slabB = slab_p.tile([128, 1024], F32, tag="slab", name="chB")
                p0 = psy_p.tile([128, JW], F32, tag="psy", name="chp0")
                p1 = psy_p.tile([128, JW], F32, tag="psy", name="chp1")
                pj = pj_p.tile([128, JW], F32, tag="pj", name="chpj")
                bc = bc_p.tile([128, JW], F32, tag="bc", name="chbc")
                banks = [slabA[:, 0:512], slabA[:, 512:1024],
                         slabB[:, 0:512], slabB[:, 512:1024],
                         p0[:], p1[:], pj[:], bc[:]]
                for ct in range(8):
                    se = dict(start=(ct == 0), stop=(ct == 7))
                    for spec, acc in zip(group, banks):
                        if spec[0] == "v":
                            nc.tensor.matmul(acc, xv(ct, spec[1]), wvt(ct), **se)
                        else:
                            _, jt, tt = spec
                            nc.tensor.matmul(acc, wqkt(ct, jt), xq(ct, tt), **se)
                for n, (spec, acc) in enumerate(zip(group, banks)):
                    # alternate copy engines so the next group's PSUM
                    # buffers free up twice as fast
                    if spec[0] == "v":
                        dst = vt(spec[1]).rearrange(
                            "p (h d) -> p h d", h=HG, d=65)[:, :, 0:64]
                        src = acc.rearrange("p (h d) -> p h d", h=HG, d=64)
                        if n % 2 == 0:
                            nc.vector.tensor_copy(dst, src)
                        else:
                            nc.scalar.copy(dst, src)
                    else:
                        _, jt, tt = spec
                        dst = qk_sb[jt][:, JW * tt:JW * tt + JW]
                        if n % 2 == 0:
                            nc.vector.tensor_copy(dst, acc)
                        else:
                            nc.scalar.copy(dst, acc)

            # ---- filler work queue (deadline-ordered): everything not done
            # by the chase, emitted into odd slab slots during attention ----
            # deadline: ("v", it) -> it // 4 ; ("p", jt, tt) -> tt ; outs -> 4
            work = deque()
            work += [("p", jt, 1) for jt in (0, 4, 1, 5, 2, 6, 3, 7)]
            work += [("v", 8), ("v", 9), ("v", 10), ("v", 11)]
            work += [("p", jt, 2) for jt in (0, 4, 1, 5, 2, 6, 3, 7)]
            work += [("v", 12), ("v", 13), ("v", 14), ("v", 15)]
            work += [("p", jt, 3) for jt in (0, 4, 1, 5, 2, 6, 3, 7)]

            def deadline(item):
                if item[0] == "v":
                    return item[1] // 4
                if item[0] == "p":
                    return item[2]
                return 4

            n_emit = [0]
            rotpool = [pj_p, bc_p]
            rottag = ["pj", "bc"]
            ot_tiles = {}

            def emit_item(item, pool=None, on_act=False):
                if pool is None:
                    k = n_emit[0] % 2
                    pool, tag = rotpool[k], rottag[k]
                else:
                    tag = {id(slab_p): "slab", id(pj_p): "pj",
                           id(bc_p): "bc"}[id(pool)]
                n_emit[0] += 1
                if item[0] == "p":
                    _, jt, tt = item
                    ps = pool.tile([128, JW], F32, tag=tag, name="psqk")
                    for ct in range(8):
                        nc.tensor.matmul(ps[:], wqkt(ct, jt), xq(ct, tt),
                                         start=(ct == 0), stop=(ct == 7))
                    nc.vector.tensor_copy(qk_sb[jt][:, JW * tt:JW * tt + JW], ps[:])
                elif item[0] == "v":
                    it = item[1]
                    ps = pool.tile([128, JW], F32, tag=tag, name="psv")
                    for ct in range(8):
                        nc.tensor.matmul(ps[:], xv(ct, it), wvt(ct),
                                         start=(ct == 0), stop=(ct == 7))
                    nc.vector.tensor_copy(
                        vt(it).rearrange("p (h d) -> p h d", h=HG, d=65)[:, :, 0:64],
                        ps[:].rearrange("p (h d) -> p h d", h=HG, d=64))
                else:
                    _, it, et = item
                    if it not in ot_tiles:
                        ot_tiles[it] = otp.tile([128, 1024], BF, tag="ot", name="ot")
                    ot = ot_tiles[it]
                    ps = pool.tile([128, JW], F32, tag=tag, name="psout")
                    for jt in range(4):
                        nc.tensor.matmul(ps[:], y_sb[jt][:, 128 * it:128 * it + 128],
                                         woutt(jt, et), start=(jt == 0), stop=(jt == 3))
                    dst = ot[:, 512 * et:512 * et + 512]
                    if on_act:
                        nc.scalar.copy(dst, ps[:])
                    else:
                        nc.vector.tensor_copy(dst, ps[:])
                    if et == 1:
                        nc.sync.dma_start(out[128 * it:128 * it + 128, :], ot[:])

            # ---- attention: J-outer, m-inner ----
            pending_norm = [None]

            def emit_norm(final=False):
                # rowsum row 64 -> reciprocal broadcast -> y^T; deferred into
                # the NEXT block so the bc matmul never heads the PE queue
                # while its rsr input is still in flight on DVE. The final
                # norm is on the critical path to the drain, so its two
                # halves run on different engines (DVE + GpSimd) in parallel
                pm, pJ, ppsy = pending_norm[0]
                pending_norm[0] = None
                rsrs = {}
                for off in (0, 1):
                    rsr = small.tile([1, JW], BF, tag="rsr", name="rsr")
                    nc.vector.tensor_copy(rsr[:], ppsy[off][64:65, :])
                    rsrs[off] = rsr
                bc = bc_p.tile([128, JW], F32, tag="bc", name="bc")
                nc.tensor.matmul(bc[0:64, :], sel_sb[0:1, 0:64], rsrs[0][:],
                                 start=True, stop=True)
                nc.tensor.matmul(bc[64:128, :], sel_sb[0:1, 0:64], rsrs[1][:],
                                 start=True, stop=True, tile_position=(0, 64))
                rec = small.tile([128, JW], F32, tag="rec", name="rec")
                nc.vector.reciprocal_approx_fast(rec[:], bc[:])
                for off in (0, 1):
                    nc.vector.tensor_mul(
                        y_sb[pm][64 * off:64 * off + 64, JW * pJ:JW * pJ + JW],
                        ppsy[off][0:64, :], rec[64 * off:64 * off + 64, :])
                if pm == 3:
                    for it in range(4 * pJ, 4 * pJ + 4):
                        work.append(("o", it, 0))
                        work.append(("o", it, 1))

            for J in range(NT):
                for m in range(4):
                    # deadline safety net: anything needed for this J must
                    # be emitted before its first slab
                    while work and deadline(work[0]) <= J:
                        emit_item(work.popleft())
                    nki = 4 * J + 4
                    last = (J == 3 and m == 3)
                    psy = {off: psy_p.tile([128, JW], F32, tag="psy",
                                           name=f"psy{off}")
                           for off in (0, 1)}
                    pvq = []

                    def emit_pv(entry, psy=psy, nki=nki, m=m):
                        pi, plo, pP = entry
                        for off in (0, 1):
                            nc.tensor.matmul(
                                psy[off][0:65, plo:JW],
                                vt(pi)[:, 65 * (2 * m + off):65 * (2 * m + off) + 65],
                                pP[:, 512 * off + plo:512 * off + 512],
                                start=(pi == 0), stop=(pi == nki - 1))

                    for i in range(nki):
                        r = i - 4 * J
                        lo = 128 * r if r > 0 else 0
                        slab = slab_p.tile([128, 1024], F32, tag="slab", name="slab")
                        for off in (0, 1):
                            nc.tensor.matmul(
                                slab[:, 512 * off + lo:512 * off + 512],
                                qk_sb[4 + m][64 * off:64 * off + 64,
                                             128 * i:128 * i + 128],
                                qk_sb[m][64 * off:64 * off + 64,
                                         JW * J + lo:JW * J + JW],
                                start=True, stop=True)
                        P = ppool.tile([128, 1024], BF, tag="p", name="P")
                        if lo:
                            nc.scalar.activation(
                                P[:].rearrange("p (o c) -> p o c", o=2)[:, :, lo:],
                                slab[:].rearrange("p (o c) -> p o c", o=2)[:, :, lo:],
                                EXP, scale=0.125)
                        else:
                            nc.scalar.activation(P[:], slab[:], EXP, scale=0.125)
                        if r >= 0:
                            for off in (0, 1):
                                blk = P[:, 512 * off + lo:512 * off + lo + 128]
                                nc.gpsimd.tensor_mul(blk, blk, tri_sb[:])
                        if i == 1 and pending_norm[0] is not None:
                            emit_norm()
                        # PV lagged 3 slabs: by emission time its exp (and the
                        # previous block's normalize, for PV(0)) are long done
                        pvq.append((i, lo, P))
                        if len(pvq) > 3:
                            emit_pv(pvq.pop(0))
                        # filler: proj/V chunks any odd slot; out chunks
                        # rate-limited so ~6 remain as filler for the last
                        # (ACT-bound) block, keeping the PE streaming there
                        if work and i % 2 == 1:
                            if deadline(work[0]) <= 3:
                                emit_item(work.popleft())
                            elif (last and i < 9) or \
                                    (i % 4 == 1 if J < 3 else i % 8 == 1):
                                emit_item(work.popleft())
                    if not last:
                        for entry in pvq:
                            emit_pv(entry)
                        pending_norm[0] = (m, J, psy)
                        continue
                    # ---- finale: the trimmed PV naturally stops writing
                    # psy cols [0:256) after pi=13, so the tq-half-A norm
                    # and its out-projection chunks can drain while PV
                    # 14/15 still accumulate half B. Held-back filler
                    # chunks cover the norm-chain latencies so the PE
                    # streams through the tail. ----
                    held = []
                    while work and len(held) < 2:
                        held.append(work.popleft())
                    emit_pv(pvq.pop(0))          # pi=13: last write to half A
                    rsA, rsB = {}, {}
                    for off in (0, 1):           # A rowsums (cols 0:256)
                        r = small.tile([1, 256], BF, tag="rsr", name="rsrA")
                        nc.vector.tensor_copy(r[:], psy[off][64:65, 0:256])
                        rsA[off] = r
                    for entry in pvq:            # pi=14,15 (cols 256+)
                        emit_pv(entry)
                    if held:                     # PE filler over rsrA latency
                        emit_item(held.pop(0), pool=pj_p)
                    bcA = bc_p.tile([128, JW], F32, tag="bc", name="bcA")
                    nc.tensor.matmul(bcA[0:64, 0:256], sel_sb[0:1, 0:64],
                                     rsA[0][:], start=True, stop=True)
                    nc.tensor.matmul(bcA[64:128, 0:256], sel_sb[0:1, 0:64],
                                     rsA[1][:], start=True, stop=True,
                                     tile_position=(0, 64))
                    if held:                     # PE filler over recA/multA
                        emit_item(held.pop(0), pool=pj_p)
                    recA = small.tile([128, JW], F32, tag="rec", name="recA")
                    nc.vector.reciprocal_approx_fast(recA[:, 0:256],
                                                     bcA[:, 0:256])
                    for off in (0, 1):
                        nc.vector.tensor_mul(
                            y_sb[3][64 * off:64 * off + 64, 1536:1792],
                            psy[off][0:64, 0:256], recA[64 * off:64 * off + 64,
                                                        0:256])
                    for off in (0, 1):           # B rowsums (cols 256:512)
                        r = small.tile([1, 256], BF, tag="rsr", name="rsrB")
                        nc.vector.tensor_copy(r[:], psy[off][64:65, 256:JW])
                        rsB[off] = r
                    emit_item(("o", 12, 0), pool=slab_p)  # covers rsrB/bcB
                    bcB = bc_p.tile([128, JW], F32, tag="bc", name="bcB")
                    nc.tensor.matmul(bcB[0:64, 0:256], sel_sb[0:1, 0:64],
                                     rsB[0][:], start=True, stop=True)
                    nc.tensor.matmul(bcB[64:128, 0:256], sel_sb[0:1, 0:64],
                                     rsB[1][:], start=True, stop=True,
                                     tile_position=(0, 64))
                    emit_item(("o", 12, 1), pool=slab_p)  # covers recB
                    recB = small.tile([128, JW], F32, tag="rec", name="recB")
                    nc.vector.reciprocal_approx_fast(recB[:, 0:256],
                                                     bcB[:, 0:256])
                    # it13 needs only normA's y columns: emit before the
                    # normB mults so it isn't false-gated on them
                    emit_item(("o", 13, 0), pool=pj_p)
                    emit_item(("o", 13, 1), pool=pj_p)
                    for off in (0, 1):
                        nc.vector.tensor_mul(
                            y_sb[3][64 * off:64 * off + 64, 1792:2048],
                            psy[off][0:64, 256:JW], recB[64 * off:64 * off + 64,
                                                         0:256])
                    for n, it in enumerate((14, 15)):
                        emit_item(("o", it, 0), pool=(slab_p, bc_p)[n],
                                  on_act=True)
                        emit_item(("o", it, 1), pool=(pj_p, slab_p)[n])
            # drain remaining out-projection chunks round-robin over psum
            # banks with copies split across ACT/DVE so nothing serializes
            drain_pools = [pj_p, slab_p, bc_p, slab_p]
            for n, item in enumerate(work):
                emit_item(item, pool=drain_pools[n % 4], on_act=(n % 2 == 0))
    nc.compile()
    return nc


def _host_trimask():
    p = np.arange(128, dtype=np.int64)[:, None]
    c = np.arange(128, dtype=np.int64)[None, :]
    return (c >= p).astype(np.float32).astype(BF16NP)


def _host_sel():
    s = np.zeros((2, 128), np.float32)
    s[0, 0:64] = 1.0
    s[1, 64:128] = 1.0
    return s.astype(BF16NP)


def _make_in_map(core, x, w_qkv, w_out):
    b, g = divmod(core, 2)
    xT = np.ascontiguousarray(x[b].T).astype(BF16NP)
    wqk = np.ascontiguousarray(np.concatenate(
        [w_qkv[:, 512 * g:512 * g + 512],
         w_qkv[:, 1024 + 512 * g:1024 + 512 * g + 512]], axis=1)).astype(BF16NP)
    wv = np.ascontiguousarray(
        w_qkv[:, 2048 + 512 * g:2048 + 512 * g + 512]).astype(BF16NP)
    wout_s = np.ascontiguousarray(w_out[512 * g:512 * g + 512, :]).astype(BF16NP)
    return dict(xT=xT, wqk=wqk, wv=wv, wout=wout_s,
                trimask=_host_trimask(), sel=_host_sel())


def kernel(x, w_qkv, w_out):
    x = np.ascontiguousarray(x, dtype=np.float32)
    w_qkv = np.ascontiguousarray(w_qkv, dtype=np.float32)
    w_out = np.ascontiguousarray(w_out, dtype=np.float32)

    if "nc" not in _cache:
        _cache["nc"] = _build()
    nc = _cache["nc"]

    in_maps = [_make_in_map(core, x, w_qkv, w_out) for core in range(8)]

    res = run_bass_kernel_spmd(nc, in_maps, core_ids=list(range(8)))
    out = np.empty((B, T, C), np.float32)
    for b in range(B):
        out[b] = (np.asarray(res.results[2 * b]["out"]).astype(np.float32)
                  + np.asarray(res.results[2 * b + 1]["out"]).astype(np.float32))
    return out


# revision 28
# speedup vs baseline: 1.0399x; 1.0399x over previous
"""Causal self-attention (B=4, T=2048, C=1024, H=16, Dh=64) on 8 trn2 NeuronCores.

Sharding: core = 2*b + g  (b = batch 0..3, g = head-group 0..1, 8 heads each).
Each core computes its batch's QKV projection for its 8 heads, causal
attention, and a partial out-projection; host sums the two head-group
partials per batch (the "all-reduce" of the tensor-parallel split).

v2 scheduling (vs v1): the whole kernel is organized so the PE never
idles and the tail never crams:
  - inputs split into per-ct (128-row) DMAs issued in need order across
    all five engine queues, so the first chase matmuls start ~10us in
    instead of ~24us (whole-tensor DMAs made the chase wait on full wv).
  - startup "chase": two groups of 8 parallel PSUM accumulation chains
    (V-proj + q/k tt0/tt1 chunks) consume each xt ct-tile as it lands.
  - attention runs J-outer / m-inner (v1 was m-outer), so the out
    projection for tq block J unlocks after J's last head pair instead
    of after 75% of the kernel; out-proj chunks become PE filler spread
    through the attention phase, shrinking the drain tail.
  - one K=2 selector matmul broadcasts both head rowsums per block
    (v1: two K=1 matmuls), halving norm PE rows.
Everything else (bf16 matmuls, ones-column rowsum in PV, trimmed exp
slabs, GpSimd triangular masking, PV lagged 3 slabs, deferred norm) is
as in v1.
"""

import sys

for _p in ("/opt/trn_rl_repo", "/opt/pypackages"):
    if _p not in sys.path:
        sys.path.append(_p)

import numpy as np
from collections import deque
from contextlib import ExitStack

import concourse.bass as bass
import concourse.tile as tile
from concourse import bacc, mybir
from concourse.bass_utils import run_bass_kernel_spmd

import ml_dtypes

BF16NP = np.dtype(ml_dtypes.bfloat16)

B, T, C = 4, 2048, 1024
H, DH = 16, 64
HG = 8          # heads per core
JW = 512        # tq tile width
NT = T // JW    # 4 tq tiles
NK = T // 128   # 16 tk tiles
F32 = mybir.dt.float32
BF = mybir.dt.bfloat16
EXP = mybir.ActivationFunctionType.Exp

_cache = {}


def _build():
    nc = bacc.Bacc("TRN2", target_bir_lowering=False, debug=False, num_devices=8)
    xT = nc.dram_tensor("xT", [C, T], BF, kind="ExternalInput").ap()
    wqk = nc.dram_tensor("wqk", [C, 1024], BF, kind="ExternalInput").ap()
    wv = nc.dram_tensor("wv", [C, 512], BF, kind="ExternalInput").ap()
    wout = nc.dram_tensor("wout", [512, C], BF, kind="ExternalInput").ap()
    trimask = nc.dram_tensor("trimask", [128, 128], BF, kind="ExternalInput").ap()
    sel = nc.dram_tensor("sel", [2, 128], BF, kind="ExternalInput").ap()
    out = nc.dram_tensor("out", [T, C], BF, kind="ExternalOutput").ap()

    with tile.TileContext(nc) as tc:
        with ExitStack() as ctx:
            ctx.enter_context(nc.allow_low_precision(reason="bf16 matmuls intended"))
            sb = ctx.enter_context(tc.tile_pool(name="sb", bufs=1))
            ppool = ctx.enter_context(tc.tile_pool(name="ppool", bufs=6))
            small = ctx.enter_context(tc.tile_pool(name="small", bufs=2))
            otp = ctx.enter_context(tc.tile_pool(name="otp", bufs=2))
            # PSUM: slab 2x2 banks + psy 2 + bc 1 + pj 1 = 8 banks exactly
            slab_p = ctx.enter_context(tc.tile_pool(name="slab_p", bufs=2, space="PSUM"))
            psy_p = ctx.enter_context(tc.tile_pool(name="psy_p", bufs=2, space="PSUM"))
            bc_p = ctx.enter_context(tc.tile_pool(name="bc_p", bufs=1, space="PSUM"))
            pj_p = ctx.enter_context(tc.tile_pool(name="pj_p", bufs=1, space="PSUM"))

            # ---- persistent SBUF (per-ct tiles: DMA dependency is per tile,
            # so consumers start as soon as their 128-row slice lands) ----
            # xt per ct, split into t-halves: the chase (V it0-7, q/k
            # tt0/tt1) reads only half A, so it lands ~7us sooner than a
            # whole [128, 2048] tile would
            xtA = [sb.tile([128, T // 2], BF, tag=f"xtA{ct}", name=f"xtA{ct}")
                   for ct in range(8)]
            xtB = [sb.tile([128, T // 2], BF, tag=f"xtB{ct}", name=f"xtB{ct}")
                   for ct in range(8)]
            wv_t = [sb.tile([128, 512], BF, tag=f"wv{ct}", name=f"wv{ct}")
                    for ct in range(8)]
            wqk_t = [sb.tile([128, 1024], BF, tag=f"wqk{ct}", name=f"wqk{ct}")
                     for ct in range(8)]
            wout_t = [sb.tile([128, 1024], BF, tag=f"wout{jt}", name=f"wout{jt}")
                      for jt in range(4)]
            qk_sb = [sb.tile([128, T], BF, tag=f"qk{j}", name=f"qk{j}") for j in range(8)]
            v_all = sb.tile([128, NK * HG * 65], BF, tag="v")
            y_sb = [sb.tile([128, T], BF, tag=f"y{m}", name=f"y{m}") for m in range(4)]
            tri_sb = sb.tile([128, 128], BF, tag="tri")
            sel_sb = sb.tile([2, 128], BF, tag="sel")
            scratch = sb.tile([1, 64], BF, tag="scratch")

            def xv(ct, it):  # V-proj stationary slice (128 t-cols)
                t = xtA[ct] if it < 8 else xtB[ct]
                return t[:, 128 * (it % 8):128 * (it % 8) + 128]

            def xq(ct, tt):  # q/k-proj moving slice (JW t-cols)
                t = xtA[ct] if tt < 2 else xtB[ct]
                return t[:, JW * (tt % 2):JW * (tt % 2) + JW]

            wvt = lambda ct: wv_t[ct][:]
            wqkt = lambda ct, jt: wqk_t[ct][:, 128 * jt:128 * jt + 128]
            woutt = lambda jt, et: wout_t[jt][:, 512 * et:512 * et + 512]
            vt = lambda i: v_all[:, 520 * i:520 * (i + 1)]

            # ---- input DMAs: need-ordered across the 3 DMA-capable queues
            # (sync/SP, scalar/ACT, gpsimd). Service is roughly FIFO by
            # issue time, so interleave wv (chase V chains) with xt pairs
            # on the two fast-starting queues; wqk (first needed ~chase
            # group G2) queues behind xt on scalar; wout + masks (needed
            # last) on gpsimd ----
            TH = T // 2
            for k in range(4):
                ct = 2 * k
                nc.sync.dma_start(xtA[ct][:], xT[128 * ct:128 * ct + 128, 0:TH])
                nc.sync.dma_start(wv_t[ct][:], wv[128 * ct:128 * ct + 128, :])
            for k in range(4):
                ct = 2 * k + 1
                nc.scalar.dma_start(xtA[ct][:], xT[128 * ct:128 * ct + 128, 0:TH])
                nc.scalar.dma_start(wv_t[ct][:], wv[128 * ct:128 * ct + 128, :])
            for k in range(4):
                nc.sync.dma_start(xtB[2 * k][:], xT[256 * k:256 * k + 128, TH:T])
                nc.scalar.dma_start(xtB[2 * k + 1][:],
                                    xT[256 * k + 128:256 * k + 256, TH:T])
            for ct in range(8):
                nc.gpsimd.dma_start(wqk_t[ct][:], wqk[128 * ct:128 * ct + 128, :])
            nc.gpsimd.dma_start(tri_sb[:], trimask[:])
            nc.gpsimd.dma_start(sel_sb[:], sel[:])
            for jt in range(4):
                nc.gpsimd.dma_start(wout_t[jt][:], wout[128 * jt:128 * jt + 128, :])
            # preload the exp table set while DMAs run
            nc.scalar.activation(scratch[:], sel_sb[0:1, 0:64], EXP, scale=0.125)
            # only the ones-columns (65th of every 65-wide head slice) need
            # setting; strided memset is ~50x cheaper than filling all of v
            nc.vector.memset(
                v_all[:].rearrange("p (x d) -> p x d", d=65)[:, :, 64:65], 1.0)

            # ---- startup chase: two groups of 8 parallel accumulation
            # chains consume each xt ct-tile as its DMA lands (8 matmuls per
            # arriving tile), instead of the PE idling through the ~25us
            # input transfer window. V group first (needs only wv+xt, the
            # earliest arrivals); the q/k tt0 group after (needs wqk,
            # arriving behind xt). The chase covers ONLY the DMA window:
            # a longer stretch of dense full-array projection matmuls
            # trips the HAM 50%-utilization power throttle (ham type-1),
            # so the rest of the projections stay interleaved with the
            # lower-power K=64 attention slabs as filler.
            CHASE = [
                [("v", it) for it in range(0, 8)],
                [("p", jt, 0) for jt in (0, 4, 1, 5, 2, 6, 3, 7)],
            ]
            for group in CHASE:
                slabA = slab_p.tile([128, 1024], F32, tag="slab", name="chA")
                slabB = slab_p.tile([128, 1024], F32, tag="slab", name="chB")
                p0 = psy_p.tile([128, JW], F32, tag="psy", name="chp0")
                p1 = psy_p.tile([128, JW], F32, tag="psy", name="chp1")
                pj = pj_p.tile([128, JW], F32, tag="pj", name="chpj")
                bc = bc_p.tile([128, JW], F32, tag="bc", name="chbc")
                banks = [slabA[:, 0:512], slabA[:, 512:1024],
                         slabB[:, 0:512], slabB[:, 512:1024],
                         p0[:], p1[:], pj[:], bc[:]]
                for ct in range(8):
                    se = dict(start=(ct == 0), stop=(ct == 7))
                    for spec, acc in zip(group, banks):
                        if spec[0] == "v":
                            nc.tensor.matmul(acc, xv(ct, spec[1]), wvt(ct), **se)
                        else:
                            _, jt, tt = spec
                            nc.tensor.matmul(acc, wqkt(ct, jt), xq(ct, tt), **se)
                for n, (spec, acc) in enumerate(zip(group, banks)):
                    # alternate copy engines so the next group's PSUM
                    # buffers free up twice as fast
                    if spec[0] == "v":
                        dst = vt(spec[1]).rearrange(
                            "p (h d) -> p h d", h=HG, d=65)[:, :, 0:64]
                        src = acc.rearrange("p (h d) -> p h d", h=HG, d=64)
                        if n % 2 == 0:
                            nc.vector.tensor_copy(dst, src)
                        else:
                            nc.scalar.copy(dst, src)
                    else:
                        _, jt, tt = spec
                        dst = qk_sb[jt][:, JW * tt:JW * tt + JW]
                        if n % 2 == 0:
                            nc.vector.tensor_copy(dst, acc)
                        else:
                            nc.scalar.copy(dst, acc)

            # ---- filler work queue (deadline-ordered): everything not done
            # by the chase, emitted into odd slab slots during attention ----
            # deadline: ("v", it) -> it // 4 ; ("p", jt, tt) -> tt ; outs -> 4
            work = deque()
            work += [("p", jt, 1) for jt in (0, 4, 1, 5, 2, 6, 3, 7)]
            work += [("v", 8), ("v", 9), ("v", 10), ("v", 11)]
            work += [("p", jt, 2) for jt in (0, 4, 1, 5, 2, 6, 3, 7)]
            work += [("v", 12), ("v", 13), ("v", 14), ("v", 15)]
            work += [("p", jt, 3) for jt in (0, 4, 1, 5, 2, 6, 3, 7)]

            def deadline(item):
                if item[0] == "v":
                    return item[1] // 4
                if item[0] == "p":
                    return item[2]
                return 4

            n_emit = [0]
            rotpool = [pj_p, bc_p]
            rottag = ["pj", "bc"]
            ot_tiles = {}

            def emit_item(item, pool=None, on_act=False):
                if pool is None:
                    k = n_emit[0] % 2
                    pool, tag = rotpool[k], rottag[k]
                else:
                    tag = {id(slab_p): "slab", id(pj_p): "pj",
                           id(bc_p): "bc"}[id(pool)]
                n_emit[0] += 1
                if item[0] == "p":
                    _, jt, tt = item
                    ps = pool.tile([128, JW], F32, tag=tag, name="psqk")
                    for ct in range(8):
                        nc.tensor.matmul(ps[:], wqkt(ct, jt), xq(ct, tt),
                                         start=(ct == 0), stop=(ct == 7))
                    nc.vector.tensor_copy(qk_sb[jt][:, JW * tt:JW * tt + JW], ps[:])
                elif item[0] == "v":
                    it = item[1]
                    ps = pool.tile([128, JW], F32, tag=tag, name="psv")
                    for ct in range(8):
                        nc.tensor.matmul(ps[:], xv(ct, it), wvt(ct),
                                         start=(ct == 0), stop=(ct == 7))
                    nc.vector.tensor_copy(
                        vt(it).rearrange("p (h d) -> p h d", h=HG, d=65)[:, :, 0:64],
                        ps[:].rearrange("p (h d) -> p h d", h=HG, d=64))
                else:
                    _, it, et = item
                    if it not in ot_tiles:
                        ot_tiles[it] = otp.tile([128, 1024], BF, tag="ot", name="ot")
                    ot = ot_tiles[it]
                    ps = pool.tile([128, JW], F32, tag=tag, name="psout")
                    for jt in range(4):
                        nc.tensor.matmul(ps[:], y_sb[jt][:, 128 * it:128 * it + 128],
                                         woutt(jt, et), start=(jt == 0), stop=(jt == 3))
                    dst = ot[:, 512 * et:512 * et + 512]
                    if on_act:
                        nc.scalar.copy(dst, ps[:])
                    else:
                        nc.vector.tensor_copy(dst, ps[:])
                    if et == 1:
                        nc.sync.dma_start(out[128 * it:128 * it + 128, :], ot[:])

            # ---- attention: J-outer, m-inner ----
            pending_norm = [None]

            def emit_norm(final=False):
                # rowsum row 64 -> reciprocal broadcast -> y^T; deferred into
                # the NEXT block so the bc matmul never heads the PE queue
                # while its rsr input is still in flight on DVE. The final
                # norm is on the critical path to the drain, so its two
                # halves run on different engines (DVE + GpSimd) in parallel
                pm, pJ, ppsy = pending_norm[0]
                pending_norm[0] = None
                rsrs = {}
                for off in (0, 1):
                    rsr = small.tile([1, JW], BF, tag="rsr", name="rsr")
                    nc.vector.tensor_copy(rsr[:], ppsy[off][64:65, :])
                    rsrs[off] = rsr
                bc = bc_p.tile([128, JW], F32, tag="bc", name="bc")
                nc.tensor.matmul(bc[0:64, :], sel_sb[0:1, 0:64], rsrs[0][:],
                                 start=True, stop=True)
                nc.tensor.matmul(bc[64:128, :], sel_sb[0:1, 0:64], rsrs[1][:],
                                 start=True, stop=True, tile_position=(0, 64))
                rec = small.tile([128, JW], F32, tag="rec", name="rec")
                nc.vector.reciprocal_approx_fast(rec[:], bc[:])
                for off in (0, 1):
                    nc.vector.tensor_mul(
                        y_sb[pm][64 * off:64 * off + 64, JW * pJ:JW * pJ + JW],
                        ppsy[off][0:64, :], rec[64 * off:64 * off + 64, :])
                if pm == 3:
                    for it in range(4 * pJ, 4 * pJ + 4):
                        work.append(("o", it, 0))
                        work.append(("o", it, 1))

            for J in range(NT):
                for m in range(4):
                    # deadline safety net: anything needed for this J must
                    # be emitted before its first slab
                    while work and deadline(work[0]) <= J:
                        emit_item(work.popleft())
                    nki = 4 * J + 4
                    last = (J == 3 and m == 3)
                    psy = {off: psy_p.tile([128, JW], F32, tag="psy",
                                           name=f"psy{off}")
                           for off in (0, 1)}
                    pvq = []

                    def emit_pv(entry, psy=psy, nki=nki, m=m):
                        pi, plo, pP = entry
                        for off in (0, 1):
                            nc.tensor.matmul(
                                psy[off][0:65, plo:JW],
                                vt(pi)[:, 65 * (2 * m + off):65 * (2 * m + off) + 65],
                                pP[:, 512 * off + plo:512 * off + 512],
                                start=(pi == 0), stop=(pi == nki - 1))

                    for i in range(nki):
                        r = i - 4 * J
                        lo = 128 * r if r > 0 else 0
                        slab = slab_p.tile([128, 1024], F32, tag="slab", name="slab")
                        for off in (0, 1):
                            nc.tensor.matmul(
                                slab[:, 512 * off + lo:512 * off + 512],
                                qk_sb[4 + m][64 * off:64 * off + 64,
                                             128 * i:128 * i + 128],
                                qk_sb[m][64 * off:64 * off + 64,
                                         JW * J + lo:JW * J + JW],
                                start=True, stop=True)
                        P = ppool.tile([128, 1024], BF, tag="p", name="P")
                        if lo:
                            nc.scalar.activation(
                                P[:].rearrange("p (o c) -> p o c", o=2)[:, :, lo:],
                                slab[:].rearrange("p (o c) -> p o c", o=2)[:, :, lo:],
                                EXP, scale=0.125)
                        else:
                            nc.scalar.activation(P[:], slab[:], EXP, scale=0.125)
                        if r >= 0:
                            for off in (0, 1):
                                blk = P[:, 512 * off + lo:512 * off + lo + 128]
                                nc.gpsimd.tensor_mul(blk, blk, tri_sb[:])
                        if i == 1 and pending_norm[0] is not None:
                            emit_norm()
                        # PV lagged 3 slabs: by emission time its exp (and the
                        # previous block's normalize, for PV(0)) are long done
                        pvq.append((i, lo, P))
                        if len(pvq) > 3:
                            emit_pv(pvq.pop(0))
                        # filler: proj/V chunks any odd slot; out chunks
                        # rate-limited so ~6 remain as filler for the last
                        # (ACT-bound) block, keeping the PE streaming there
                        if work and i % 2 == 1:
                            if deadline(work[0]) <= 3:
                                emit_item(work.popleft())
                            elif (last and i < 9) or \
                                    (i % 4 == 1 if J < 3 else i % 8 == 1):
                                emit_item(work.popleft())
                    for entry in pvq:
                        emit_pv(entry)
                    pending_norm[0] = (m, J, psy)
            # ---- final norm, with held-back out chunks woven through its
            # latency so the PE keeps streaming, and its two normalize
            # mults split across DVE / GpSimd (via an ACT-staged SBUF copy;
            # GpSimd can't read PSUM) ----
            held = []
            while work and len(held) < 2:
                held.append(work.popleft())
            pm, pJ, ppsy = pending_norm[0]
            pending_norm[0] = None
            psy1s = small.tile([128, JW], F32, tag="psy1s", name="psy1s")
            nc.scalar.copy(psy1s[64:128, :], ppsy[1][0:64, :])
            rsr0 = small.tile([1, JW], BF, tag="rsr", name="rsr0")
            nc.vector.tensor_copy(rsr0[:], ppsy[0][64:65, :])
            rsr1 = small.tile([1, JW], BF, tag="rsr", name="rsr1")
            nc.scalar.copy(rsr1[:], ppsy[1][64:65, :])
            if held:
                emit_item(held.pop(0), pool=pj_p)
            bc = bc_p.tile([128, JW], F32, tag="bc", name="bcF")
            nc.tensor.matmul(bc[0:64, :], sel_sb[0:1, 0:64], rsr0[:],
                             start=True, stop=True)
            nc.tensor.matmul(bc[64:128, :], sel_sb[0:1, 0:64], rsr1[:],
                             start=True, stop=True, tile_position=(0, 64))
            if held:
                emit_item(held.pop(0), pool=pj_p)
            rec = small.tile([128, JW], F32, tag="rec", name="recF")
            nc.vector.reciprocal_approx_fast(rec[:], bc[:])
            nc.vector.tensor_mul(y_sb[pm][0:64, JW * pJ:JW * pJ + JW],
                                 ppsy[0][0:64, :], rec[0:64, :])
            nc.gpsimd.tensor_mul(y_sb[pm][64:128, JW * pJ:JW * pJ + JW],
                                 psy1s[64:128, :], rec[64:128, :])
            for it in range(4 * pJ, 4 * pJ + 4):
                work.append(("o", it, 0))
                work.append(("o", it, 1))
            # drain remaining out-projection chunks round-robin over psum
            # banks with copies split across ACT/DVE so nothing serializes
            drain_pools = [pj_p, slab_p, bc_p, slab_p]
            for n, item in enumerate(work):
                emit_item(item, pool=drain_pools[n % 4], on_act=(n % 2 == 0))
    nc.compile()
    return nc


def _host_trimask():
    p = np.arange(128, dtype=np.int64)[:, None]
    c = np.arange(128, dtype=np.int64)[None, :]
    return (c >= p).astype(np.float32).astype(BF16NP)


def _host_sel():
    s = np.zeros((2, 128), np.float32)
    s[0, 0:64] = 1.0
    s[1, 64:128] = 1.0
    return s.astype(BF16NP)


def _make_in_map(core, x, w_qkv, w_out):
    b, g = divmod(core, 2)
    xT = np.ascontiguousarray(x[b].T).astype(BF16NP)
    wqk = np.ascontiguousarray(np.concatenate(
        [w_qkv[:, 512 * g:512 * g + 512],
         w_qkv[:, 1024 + 512 * g:1024 + 512 * g + 512]], axis=1)).astype(BF16NP)
    wv = np.ascontiguousarray(
        w_qkv[:, 2048 + 512 * g:2048 + 512 * g + 512]).astype(BF16NP)
    wout_s = np.ascontiguousarray(w_out[512 * g:512 * g + 512, :]).astype(BF16NP)
    return dict(xT=xT, wqk=wqk, wv=wv, wout=wout_s,
                trimask=_host_trimask(), sel=_host_sel())


def kernel(x, w_qkv, w_out):
    x = np.ascontiguousarray(x, dtype=np.float32)
    w_qkv = np.ascontiguousarray(w_qkv, dtype=np.float32)
    w_out = np.ascontiguousarray(w_out, dtype=np.float32)

    if "nc" not in _cache:
        _cache["nc"] = _build()
    nc = _cache["nc"]

    in_maps = [_make_in_map(core, x, w_qkv, w_out) for core in range(8)]

    res = run_bass_kernel_spmd(nc, in_maps, core_ids=list(range(8)))
    out = np.empty((B, T, C), np.float32)
    for b in range(B):
        out[b] = (np.asarray(res.results[2 * b]["out"]).astype(np.float32)
                  + np.asarray(res.results[2 * b + 1]["out"]).astype(np.float32))
    return out


# revision 32
# speedup vs baseline: 1.0425x; 1.0025x over previous
"""Causal self-attention (B=4, T=2048, C=1024, H=16, Dh=64) on 8 trn2 NeuronCores.

Sharding: core = 2*b + g  (b = batch 0..3, g = head-group 0..1, 8 heads each).
Each core computes its batch's QKV projection for its 8 heads, causal
attention, and a partial out-projection; host sums the two head-group
partials per batch (the "all-reduce" of the tensor-parallel split).

v2 scheduling (vs v1): the whole kernel is organized so the PE never
idles and the tail never crams:
  - inputs split into per-ct (128-row) DMAs issued in need order across
    all five engine queues, so the first chase matmuls start ~10us in
    instead of ~24us (whole-tensor DMAs made the chase wait on full wv).
  - startup "chase": two groups of 8 parallel PSUM accumulation chains
    (V-proj + q/k tt0/tt1 chunks) consume each xt ct-tile as it lands.
  - attention runs J-outer / m-inner (v1 was m-outer), so the out
    projection for tq block J unlocks after J's last head pair instead
    of after 75% of the kernel; out-proj chunks become PE filler spread
    through the attention phase, shrinking the drain tail.
  - one K=2 selector matmul broadcasts both head rowsums per block
    (v1: two K=1 matmuls), halving norm PE rows.
Everything else (bf16 matmuls, ones-column rowsum in PV, trimmed exp
slabs, GpSimd triangular masking, PV lagged 3 slabs, deferred norm) is
as in v1.
"""

import sys

for _p in ("/opt/trn_rl_repo", "/opt/pypackages"):
    if _p not in sys.path:
        sys.path.append(_p)

import numpy as np
from collections import deque
from contextlib import ExitStack

import concourse.bass as bass
import concourse.tile as tile
from concourse import bacc, mybir
from concourse.bass_utils import run_bass_kernel_spmd

import ml_dtypes

BF16NP = np.dtype(ml_dtypes.bfloat16)

B, T, C = 4, 2048, 1024
H, DH = 16, 64
HG = 8          # heads per core
JW = 512        # tq tile width
NT = T // JW    # 4 tq tiles
NK = T // 128   # 16 tk tiles
F32 = mybir.dt.float32
BF = mybir.dt.bfloat16
EXP = mybir.ActivationFunctionType.Exp

_cache = {}


def _build():
    nc = bacc.Bacc("TRN2", target_bir_lowering=False, debug=False, num_devices=8)
    xT = nc.dram_tensor("xT", [C, T], BF, kind="ExternalInput").ap()
    wqk = nc.dram_tensor("wqk", [C, 1024], BF, kind="ExternalInput").ap()
    wv = nc.dram_tensor("wv", [C, 512], BF, kind="ExternalInput").ap()
    wout = nc.dram_tensor("wout", [512, C], BF, kind="ExternalInput").ap()
    trimask = nc.dram_tensor("trimask", [128, 128], BF, kind="ExternalInput").ap()
    sel = nc.dram_tensor("sel", [2, 128], BF, kind="ExternalInput").ap()
    out = nc.dram_tensor("out", [T, C], BF, kind="ExternalOutput").ap()

    with tile.TileContext(nc) as tc:
        with ExitStack() as ctx:
            ctx.enter_context(nc.allow_low_precision(reason="bf16 matmuls intended"))
            sb = ctx.enter_context(tc.tile_pool(name="sb", bufs=1))
            ppool = ctx.enter_context(tc.tile_pool(name="ppool", bufs=6))
            small = ctx.enter_context(tc.tile_pool(name="small", bufs=2))
            otp = ctx.enter_context(tc.tile_pool(name="otp", bufs=2))
            # PSUM: slab 2x2 banks + psy 2 + bc 1 + pj 1 = 8 banks exactly
            slab_p = ctx.enter_context(tc.tile_pool(name="slab_p", bufs=2, space="PSUM"))
            psy_p = ctx.enter_context(tc.tile_pool(name="psy_p", bufs=2, space="PSUM"))
            bc_p = ctx.enter_context(tc.tile_pool(name="bc_p", bufs=1, space="PSUM"))
            pj_p = ctx.enter_context(tc.tile_pool(name="pj_p", bufs=1, space="PSUM"))

            # ---- persistent SBUF (per-ct tiles: DMA dependency is per tile,
            # so consumers start as soon as their 128-row slice lands) ----
            # xt per ct, split into t-halves: the chase (V it0-7, q/k
            # tt0/tt1) reads only half A, so it lands ~7us sooner than a
            # whole [128, 2048] tile would
            xtA = [sb.tile([128, T // 2], BF, tag=f"xtA{ct}", name=f"xtA{ct}")
                   for ct in range(8)]
            xtB = [sb.tile([128, T // 2], BF, tag=f"xtB{ct}", name=f"xtB{ct}")
                   for ct in range(8)]
            wv_t = [sb.tile([128, 512], BF, tag=f"wv{ct}", name=f"wv{ct}")
                    for ct in range(8)]
            wqk_t = [sb.tile([128, 1024], BF, tag=f"wqk{ct}", name=f"wqk{ct}")
                     for ct in range(8)]
            wout_t = [sb.tile([128, 1024], BF, tag=f"wout{jt}", name=f"wout{jt}")
                      for jt in range(4)]
            qk_sb = [sb.tile([128, T], BF, tag=f"qk{j}", name=f"qk{j}") for j in range(8)]
            v_all = sb.tile([128, NK * HG * 65], BF, tag="v")
            y_sb = [sb.tile([128, T], BF, tag=f"y{m}", name=f"y{m}") for m in range(4)]
            tri_sb = sb.tile([128, 128], BF, tag="tri")
            sel_sb = sb.tile([2, 128], BF, tag="sel")
            scratch = sb.tile([1, 64], BF, tag="scratch")

            def xv(ct, it):  # V-proj stationary slice (128 t-cols)
                t = xtA[ct] if it < 8 else xtB[ct]
                return t[:, 128 * (it % 8):128 * (it % 8) + 128]

            def xq(ct, tt):  # q/k-proj moving slice (JW t-cols)
                t = xtA[ct] if tt < 2 else xtB[ct]
                return t[:, JW * (tt % 2):JW * (tt % 2) + JW]

            wvt = lambda ct: wv_t[ct][:]
            wqkt = lambda ct, jt: wqk_t[ct][:, 128 * jt:128 * jt + 128]
            woutt = lambda jt, et: wout_t[jt][:, 512 * et:512 * et + 512]
            vt = lambda i: v_all[:, 520 * i:520 * (i + 1)]

            # ---- input DMAs: need-ordered across the 3 DMA-capable queues
            # (sync/SP, scalar/ACT, gpsimd). Service is roughly FIFO by
            # issue time, so interleave wv (chase V chains) with xt pairs
            # on the two fast-starting queues; wqk (first needed ~chase
            # group G2) queues behind xt on scalar; wout + masks (needed
            # last) on gpsimd ----
            # pair (xtA_ct, wv_ct) ACROSS the two queues so both halves of a
            # ct's inputs land simultaneously (the first chase matmul needs
            # xtA0 AND wv0 — serializing them on one queue cost ~2us)
            TH = T // 2
            for ct in range(8):
                qa, qb = (nc.sync, nc.scalar) if ct % 2 == 0 else (nc.scalar, nc.sync)
                qa.dma_start(xtA[ct][:], xT[128 * ct:128 * ct + 128, 0:TH])
                qb.dma_start(wv_t[ct][:], wv[128 * ct:128 * ct + 128, :])
            for k in range(4):
                nc.sync.dma_start(xtB[2 * k][:], xT[256 * k:256 * k + 128, TH:T])
                nc.scalar.dma_start(xtB[2 * k + 1][:],
                                    xT[256 * k + 128:256 * k + 256, TH:T])
            for ct in range(8):
                nc.gpsimd.dma_start(wqk_t[ct][:], wqk[128 * ct:128 * ct + 128, :])
            nc.gpsimd.dma_start(tri_sb[:], trimask[:])
            nc.gpsimd.dma_start(sel_sb[:], sel[:])
            for jt in range(4):
                nc.gpsimd.dma_start(wout_t[jt][:], wout[128 * jt:128 * jt + 128, :])
            # preload the exp table set while DMAs run
            nc.scalar.activation(scratch[:], sel_sb[0:1, 0:64], EXP, scale=0.125)
            # only the ones-columns (65th of every 65-wide head slice) need
            # setting; strided memset is ~50x cheaper than filling all of v
            nc.vector.memset(
                v_all[:].rearrange("p (x d) -> p x d", d=65)[:, :, 64:65], 1.0)

            # ---- startup chase: two groups of 8 parallel accumulation
            # chains consume each xt ct-tile as its DMA lands (8 matmuls per
            # arriving tile), instead of the PE idling through the ~25us
            # input transfer window. V group first (needs only wv+xt, the
            # earliest arrivals); the q/k tt0 group after (needs wqk,
            # arriving behind xt). The chase covers ONLY the DMA window:
            # a longer stretch of dense full-array projection matmuls
            # trips the HAM 50%-utilization power throttle (ham type-1),
            # so the rest of the projections stay interleaved with the
            # lower-power K=64 attention slabs as filler.
            CHASE = [
                [("v", it) for it in range(0, 8)],
                [("p", jt, 0) for jt in (0, 4, 1, 5, 2, 6, 3, 7)],
            ]
            for group in CHASE:
                slabA = slab_p.tile([128, 1024], F32, tag="slab", name="chA")
                slabB = slab_p.tile([128, 1024], F32, tag="slab", name="chB")
                p0 = psy_p.tile([128, JW], F32, tag="psy", name="chp0")
                p1 = psy_p.tile([128, JW], F32, tag="psy", name="chp1")
                pj = pj_p.tile([128, JW], F32, tag="pj", name="chpj")
                bc = bc_p.tile([128, JW], F32, tag="bc", name="chbc")
                banks = [slabA[:, 0:512], slabA[:, 512:1024],
                         slabB[:, 0:512], slabB[:, 512:1024],
                         p0[:], p1[:], pj[:], bc[:]]
                for ct in range(8):
                    se = dict(start=(ct == 0), stop=(ct == 7))
                    for spec, acc in zip(group, banks):
                        if spec[0] == "v":
                            nc.tensor.matmul(acc, xv(ct, spec[1]), wvt(ct), **se)
                        else:
                            _, jt, tt = spec
                            nc.tensor.matmul(acc, wqkt(ct, jt), xq(ct, tt), **se)
                for n, (spec, acc) in enumerate(zip(group, banks)):
                    # alternate copy engines so the next group's PSUM
                    # buffers free up twice as fast
                    if spec[0] == "v":
                        dst = vt(spec[1]).rearrange(
                            "p (h d) -> p h d", h=HG, d=65)[:, :, 0:64]
                        src = acc.rearrange("p (h d) -> p h d", h=HG, d=64)
                        if n % 2 == 0:
                            nc.vector.tensor_copy(dst, src)
                        else:
                            nc.scalar.copy(dst, src)
                    else:
                        _, jt, tt = spec
                        dst = qk_sb[jt][:, JW * tt:JW * tt + JW]
                        if n % 2 == 0:
                            nc.vector.tensor_copy(dst, acc)
                        else:
                            nc.scalar.copy(dst, acc)

            # ---- filler work queue (deadline-ordered): everything not done
            # by the chase, emitted into odd slab slots during attention ----
            # deadline: ("v", it) -> it // 4 ; ("p", jt, tt) -> tt ; outs -> 4
            work = deque()
            work += [("p", jt, 1) for jt in (0, 4, 1, 5, 2, 6, 3, 7)]
            work += [("v", 8), ("v", 9), ("v", 10), ("v", 11)]
            work += [("p", jt, 2) for jt in (0, 4, 1, 5, 2, 6, 3, 7)]
            work += [("v", 12), ("v", 13), ("v", 14), ("v", 15)]
            work += [("p", jt, 3) for jt in (0, 4, 1, 5, 2, 6, 3, 7)]

            def deadline(item):
                if item[0] == "v":
                    return item[1] // 4
                if item[0] == "p":
                    return item[2]
                return 4

            n_emit = [0]
            rotpool = [pj_p, bc_p]
            rottag = ["pj", "bc"]
            ot_tiles = {}

            def emit_item(item, pool=None, on_act=False):
                if pool is None:
                    k = n_emit[0] % 2
                    pool, tag = rotpool[k], rottag[k]
                else:
                    tag = {id(slab_p): "slab", id(pj_p): "pj",
                           id(bc_p): "bc"}[id(pool)]
                n_emit[0] += 1
                if item[0] == "p":
                    _, jt, tt = item
                    ps = pool.tile([128, JW], F32, tag=tag, name="psqk")
                    for ct in range(8):
                        nc.tensor.matmul(ps[:], wqkt(ct, jt), xq(ct, tt),
                                         start=(ct == 0), stop=(ct == 7))
                    nc.vector.tensor_copy(qk_sb[jt][:, JW * tt:JW * tt + JW], ps[:])
                elif item[0] == "v":
                    it = item[1]
                    ps = pool.tile([128, JW], F32, tag=tag, name="psv")
                    for ct in range(8):
                        nc.tensor.matmul(ps[:], xv(ct, it), wvt(ct),
                                         start=(ct == 0), stop=(ct == 7))
                    nc.vector.tensor_copy(
                        vt(it).rearrange("p (h d) -> p h d", h=HG, d=65)[:, :, 0:64],
                        ps[:].rearrange("p (h d) -> p h d", h=HG, d=64))
                else:
                    _, it, et = item
                    if it not in ot_tiles:
                        ot_tiles[it] = otp.tile([128, 1024], BF, tag="ot", name="ot")
                    ot = ot_tiles[it]
                    ps = pool.tile([128, JW], F32, tag=tag, name="psout")
                    for jt in range(4):
                        nc.tensor.matmul(ps[:], y_sb[jt][:, 128 * it:128 * it + 128],
                                         woutt(jt, et), start=(jt == 0), stop=(jt == 3))
                    dst = ot[:, 512 * et:512 * et + 512]
                    if on_act:
                        nc.scalar.copy(dst, ps[:])
                    else:
                        nc.vector.tensor_copy(dst, ps[:])
                    # per-et half DMA: each half ships right after its copy,
                    # halving the final exposed DMA at kernel end
                    nc.sync.dma_start(
                        out[128 * it:128 * it + 128, 512 * et:512 * et + 512],
                        dst)

            # ---- attention: J-outer, m-inner ----
            pending_norm = [None]

            def emit_norm(final=False):
                # rowsum row 64 -> reciprocal broadcast -> y^T; deferred into
                # the NEXT block so the bc matmul never heads the PE queue
                # while its rsr input is still in flight on DVE. The final
                # norm is on the critical path to the drain, so its two
                # halves run on different engines (DVE + GpSimd) in parallel
                pm, pJ, ppsy = pending_norm[0]
                pending_norm[0] = None
                rsrs = {}
                for off in (0, 1):
                    rsr = small.tile([1, JW], BF, tag="rsr", name="rsr")
                    nc.vector.tensor_copy(rsr[:], ppsy[off][64:65, :])
                    rsrs[off] = rsr
                bc = bc_p.tile([128, JW], F32, tag="bc", name="bc")
                nc.tensor.matmul(bc[0:64, :], sel_sb[0:1, 0:64], rsrs[0][:],
                                 start=True, stop=True)
                nc.tensor.matmul(bc[64:128, :], sel_sb[0:1, 0:64], rsrs[1][:],
                                 start=True, stop=True, tile_position=(0, 64))
                rec = small.tile([128, JW], F32, tag="rec", name="rec")
                nc.vector.reciprocal_approx_fast(rec[:], bc[:])
                for off in (0, 1):
                    nc.vector.tensor_mul(
                        y_sb[pm][64 * off:64 * off + 64, JW * pJ:JW * pJ + JW],
                        ppsy[off][0:64, :], rec[64 * off:64 * off + 64, :])
                if pm == 3:
                    for it in range(4 * pJ, 4 * pJ + 4):
                        work.append(("o", it, 0))
                        work.append(("o", it, 1))

            for J in range(NT):
                for m in range(4):
                    # deadline safety net: anything needed for this J must
                    # be emitted before its first slab
                    while work and deadline(work[0]) <= J:
                        emit_item(work.popleft())
                    nki = 4 * J + 4
                    last = (J == 3 and m == 3)
                    psy = {off: psy_p.tile([128, JW], F32, tag="psy",
                                           name=f"psy{off}")
                           for off in (0, 1)}
                    pvq = []

                    def emit_pv(entry, psy=psy, nki=nki, m=m):
                        pi, plo, pP = entry
                        for off in (0, 1):
                            nc.tensor.matmul(
                                psy[off][0:65, plo:JW],
                                vt(pi)[:, 65 * (2 * m + off):65 * (2 * m + off) + 65],
                                pP[:, 512 * off + plo:512 * off + 512],
                                start=(pi == 0), stop=(pi == nki - 1))

                    for i in range(nki):
                        r = i - 4 * J
                        lo = 128 * r if r > 0 else 0
                        slab = slab_p.tile([128, 1024], F32, tag="slab", name="slab")
                        for off in (0, 1):
                            nc.tensor.matmul(
                                slab[:, 512 * off + lo:512 * off + 512],
                                qk_sb[4 + m][64 * off:64 * off + 64,
                                             128 * i:128 * i + 128],
                                qk_sb[m][64 * off:64 * off + 64,
                                         JW * J + lo:JW * J + JW],
                                start=True, stop=True)
                        P = ppool.tile([128, 1024], BF, tag="p", name="P")
                        if lo:
                            nc.scalar.activation(
                                P[:].rearrange("p (o c) -> p o c", o=2)[:, :, lo:],
                                slab[:].rearrange("p (o c) -> p o c", o=2)[:, :, lo:],
                                EXP, scale=0.125)
                        else:
                            nc.scalar.activation(P[:], slab[:], EXP, scale=0.125)
                        if r >= 0:
                            for off in (0, 1):
                                blk = P[:, 512 * off + lo:512 * off + lo + 128]
                                nc.gpsimd.tensor_mul(blk, blk, tri_sb[:])
                        if i == 1 and pending_norm[0] is not None:
                            emit_norm()
                        # PV lagged 3 slabs: by emission time its exp (and the
                        # previous block's normalize, for PV(0)) are long done
                        pvq.append((i, lo, P))
                        if len(pvq) > 3:
                            emit_pv(pvq.pop(0))
                        # filler: proj/V chunks any odd slot; out chunks
                        # rate-limited so ~6 remain as filler for the last
                        # (ACT-bound) block, keeping the PE streaming there
                        if work and i % 2 == 1:
                            if deadline(work[0]) <= 3:
                                emit_item(work.popleft())
                            elif (last and i < 9) or \
                                    (i % 4 == 1 if J < 3 else i % 8 == 1):
                                emit_item(work.popleft())
                    for entry in pvq:
                        emit_pv(entry)
                    pending_norm[0] = (m, J, psy)
            # ---- final norm, with held-back out chunks woven through its
            # latency so the PE keeps streaming, and its two normalize
            # mults split across DVE / GpSimd (via an ACT-staged SBUF copy;
            # GpSimd can't read PSUM) ----
            held = []
            while work and len(held) < 2:
                held.append(work.popleft())
            pm, pJ, ppsy = pending_norm[0]
            pending_norm[0] = None
            psy1s = small.tile([128, JW], F32, tag="psy1s", name="psy1s")
            nc.scalar.copy(psy1s[64:128, :], ppsy[1][0:64, :])
            rsr0 = small.tile([1, JW], BF, tag="rsr", name="rsr0")
            nc.vector.tensor_copy(rsr0[:], ppsy[0][64:65, :])
            rsr1 = small.tile([1, JW], BF, tag="rsr", name="rsr1")
            nc.scalar.copy(rsr1[:], ppsy[1][64:65, :])
            if held:
                emit_item(held.pop(0), pool=pj_p)
            bc = bc_p.tile([128, JW], F32, tag="bc", name="bcF")
            nc.tensor.matmul(bc[0:64, :], sel_sb[0:1, 0:64], rsr0[:],
                             start=True, stop=True)
            nc.tensor.matmul(bc[64:128, :], sel_sb[0:1, 0:64], rsr1[:],
                             start=True, stop=True, tile_position=(0, 64))
            if held:
                emit_item(held.pop(0), pool=pj_p)
            rec = small.tile([128, JW], F32, tag="rec", name="recF")
            nc.vector.reciprocal_approx_fast(rec[:], bc[:])
            nc.vector.tensor_mul(y_sb[pm][0:64, JW * pJ:JW * pJ + JW],
                                 ppsy[0][0:64, :], rec[0:64, :])
            nc.gpsimd.tensor_mul(y_sb[pm][64:128, JW * pJ:JW * pJ + JW],
                                 psy1s[64:128, :], rec[64:128, :])
            for it in range(4 * pJ, 4 * pJ + 4):
                work.append(("o", it, 0))
                work.append(("o", it, 1))
            # drain remaining out-projection chunks round-robin over psum
            # banks with copies split across ACT/DVE so nothing serializes
            drain_pools = [pj_p, slab_p, bc_p, slab_p]
            for n, item in enumerate(work):
                emit_item(item, pool=drain_pools[n % 4], on_act=(n % 2 == 0))
    nc.compile()
    return nc


def _host_trimask():
    p = np.arange(128, dtype=np.int64)[:, None]
    c = np.arange(128, dtype=np.int64)[None, :]
    return (c >= p).astype(np.float32).astype(BF16NP)


def _host_sel():
    s = np.zeros((2, 128), np.float32)
    s[0, 0:64] = 1.0
    s[1, 64:128] = 1.0
    return s.astype(BF16NP)


def _make_in_map(core, x, w_qkv, w_out):
    b, g = divmod(core, 2)
    xT = np.ascontiguousarray(x[b].T).astype(BF16NP)
    wqk = np.ascontiguousarray(np.concatenate(
        [w_qkv[:, 512 * g:512 * g + 512],
         w_qkv[:, 1024 + 512 * g:1024 + 512 * g + 512]], axis=1)).astype(BF16NP)
    wv = np.ascontiguousarray(
        w_qkv[:, 2048 + 512 * g:2048 + 512 * g + 512]).astype(BF16NP)
    wout_s = np.ascontiguousarray(w_out[512 * g:512 * g + 512, :]).astype(BF16NP)
    return dict(xT=xT, wqk=wqk, wv=wv, wout=wout_s,
                trimask=_host_trimask(), sel=_host_sel())


def kernel(x, w_qkv, w_out):
    x = np.ascontiguousarray(x, dtype=np.float32)
    w_qkv = np.ascontiguousarray(w_qkv, dtype=np.float32)
    w_out = np.ascontiguousarray(w_out, dtype=np.float32)

    if "nc" not in _cache:
        _cache["nc"] = _build()
    nc = _cache["nc"]

    in_maps = [_make_in_map(core, x, w_qkv, w_out) for core in range(8)]

    res = run_bass_kernel_spmd(nc, in_maps, core_ids=list(range(8)))
    out = np.empty((B, T, C), np.float32)
    for b in range(B):
        out[b] = (np.asarray(res.results[2 * b]["out"]).astype(np.float32)
                  + np.asarray(res.results[2 * b + 1]["out"]).astype(np.float32))
    return out
